# revision 7
# baseline (speedup 1.0000x reference)
"""MultiHeadAttention (B=2, S=2048, D=1024, H=16, softmax over query axis)
on 8 TRN2 NeuronCores.

Sharding: core c handles batch b = c//4 and head-group hg = c%4 (4 heads,
d_local = 256). QKV weights row-sharded by head group, Wo column-sharded;
each core produces a partial [S, D] output, host sums the 4 partials per
batch and adds the output bias.

Device kernel (per core), all matmuls bf16 with fp32 PSUM accumulation:
  xT [D, S] resident in SBUF (per-128-row tiles, DMA'd after the weights so
  projection matmuls pipeline with the input DMA).
  Q^T, K^T = Wl^T-stationary matmuls -> [d_local, S] (+bias, bf16)
  V = xT-stationary matmul -> [S, d_local] (+bias, bf16)
  per head: scores^T[k, q] = K^T-slice.T @ Q^T (contract d=64)
            softmax over q = free-axis Exp + accum_out row sums (fp32)
            normalizer folded into V rows: vs = V[k,:]*r[k]
            W^T[d, q] += vs.T @ exp(scores^T)   (contract k)
  out[s, j] = W^T-slices.T @ Wo^T  (contract d_local)
"""

import os

import numpy as np
import ml_dtypes

import concourse.tile as tile
from concourse import bacc, mybir
from concourse.bass_utils import run_bass_kernel_spmd

B, S, D, H = 2, 2048, 1024, 16
HD = D // H            # 64
NCORES = 8
HPC = H // (NCORES // B)   # heads per core = 4
DL = HPC * HD              # local head dims = 256
CT = D // 128              # 8 contraction tiles over D
ST = S // 128              # 16 seq tiles
BF = mybir.dt.bfloat16
F32 = mybir.dt.float32
bf16 = ml_dtypes.bfloat16

_CACHE = {}
LAST_RESULT = None


def _emit_body(nc, tc, aps):
    xT, wqT, wkT, wvT, woT, bq, bk, bv, out = aps
    with tc.tile_pool(name="const", bufs=1) as cp:
        # --- biases first (tiny), then weights+x interleaved per c-tile so
        # the c-chain of projection matmuls starts as soon as possible.
        bq_sb = cp.tile([128, 2], F32)
        bk_sb = cp.tile([128, 2], F32)
        nc.sync.dma_start(bq_sb[:], bq)
        nc.sync.dma_start(bk_sb[:], bk)
        bv_row = cp.tile([1, DL], F32)
        nc.sync.dma_start(bv_row[:], bv)
        bvb = cp.tile([128, DL], F32)
        nc.gpsimd.partition_broadcast(bvb[:], bv_row[:])

        wq_c, wk_c, wv_c, xt_ch = [], [], [], []
        for c in range(CT):
            cslice = slice(c * 128, (c + 1) * 128)
            tq = cp.tile([128, DL], BF, tag=f"wq{c}", name=f"wq{c}")
            tk = cp.tile([128, DL], BF, tag=f"wk{c}", name=f"wk{c}")
            tv = cp.tile([128, DL], BF, tag=f"wv{c}", name=f"wv{c}")
            nc.sync.dma_start(tq[:], wqT[cslice, :])
            nc.sync.dma_start(tk[:], wkT[cslice, :])
            nc.sync.dma_start(tv[:], wvT[cslice, :])
            tx = cp.tile([128, S], BF, tag=f"xt{c}", name=f"xt{c}")
            nc.sync.dma_start(tx[:], xT[cslice, :])
            xt_ch.append(tx)
            wq_c.append(tq)
            wk_c.append(tk)
            wv_c.append(tv)

        woT_sb = cp.tile([128, DL // 128, D], BF)
        nc.sync.dma_start(woT_sb[:], woT.rearrange("(c p) j -> p c j", p=128))

        qT_sb = [cp.tile([128, S], BF, tag=f"qT{m}", name=f"qT{m}")
                 for m in range(2)]
        kT_sb = [cp.tile([128, S], BF, tag=f"kT{m}", name=f"kT{m}")
                 for m in range(2)]
        v_sb = cp.tile([128, ST, DL], BF)
        wt_sb = [cp.tile([128, S], BF, tag=f"wt{m}", name=f"wt{m}")
                 for m in range(2)]

        # ---- QKV projections + attention, overlapped ----
        # The scores psum pool opens BEFORE the projection pool (LIFO stack)
        # so head 0's scores+exp chain starts right after the m0 projections
        # while the m1/V projections still run on PE; head 0's A*V matmuls
        # are deferred (E tiles retained) until V exists and the projection
        # pool's banks have been handed to the W accumulator pool.
        def scores_exp(spsp, ep, sp, kTh, qTh, kt, sacc_tag=""):
            e_half = []
            for qh in range(2):
                sps = spsp.tile([128, 1024], F32, tag="sps")
                for ch in range(2):
                    q0 = qh * 1024 + ch * 512
                    nc.tensor.matmul(
                        sps[:, ch * 512:(ch + 1) * 512],
                        kTh[:, kt * 128:(kt + 1) * 128],
                        qTh[:, q0:q0 + 512],
                        start=True, stop=True)
                e_t = ep.tile([128, 1024], BF, tag="E")
                sacc = sp.tile([128, 1], F32, tag=f"sacc{qh}{sacc_tag}")
                nc.scalar.activation(
                    e_t[:], sps[:], mybir.ActivationFunctionType.Exp,
                    scale=float(1.0 / np.sqrt(HD)), accum_out=sacc[:])
                e_half.append((e_t, sacc))
            return e_half

        def norm_vs(sp, e_half, kt, vcol, tag=""):
            stot = sp.tile([128, 1], F32, tag=f"stot{tag}")
            nc.vector.tensor_add(stot[:], e_half[0][1][:], e_half[1][1][:])
            r = sp.tile([128, 1], F32, tag=f"r{tag}")
            nc.vector.reciprocal(r[:], stot[:])
            vs = sp.tile([128, HD], BF, tag=f"vs{tag}")
            nc.vector.tensor_scalar_mul(
                vs[:], v_sb[:, kt, vcol:vcol + HD], r[:])
            return vs

        def av(wps, vs, e_half, kt, off, tp):
            for qh in range(2):
                for ch in range(2):
                    q0 = qh * 1024 + ch * 512
                    nc.tensor.matmul(
                        wps[off:off + 64, q0:q0 + 512],
                        vs[:], e_half[qh][0][:, ch * 512:(ch + 1) * 512],
                        start=(kt == 0), stop=(kt == ST - 1),
                        tile_position=tp)

        def proj_qk_chunk(qps, wc, bsb, dst, m, ch):
            pq = qps.tile([128, 512], F32, tag="pq")
            for c in range(CT):
                nc.tensor.matmul(
                    pq[:],
                    wc[c][:, m * 128:(m + 1) * 128],
                    xt_ch[c][:, ch * 512:(ch + 1) * 512],
                    start=(c == 0), stop=(c == CT - 1))
            nc.vector.tensor_scalar_add(
                dst[m][:, ch * 512:(ch + 1) * 512], pq[:], bsb[:, m:m + 1])

        def proj_qk(qps, wc, bsb, dst, m):
            for ch in range(4):
                pq = qps.tile([128, 512], F32, tag="pq")
                for c in range(CT):
                    nc.tensor.matmul(
                        pq[:],
                        wc[c][:, m * 128:(m + 1) * 128],
                        xt_ch[c][:, ch * 512:(ch + 1) * 512],
                        start=(c == 0), stop=(c == CT - 1))
                nc.vector.tensor_scalar_add(
                    dst[m][:, ch * 512:(ch + 1) * 512], pq[:],
                    bsb[:, m:m + 1])

        with tc.tile_pool(name="att_sb", bufs=36) as ep, \
             tc.tile_pool(name="small", bufs=4) as sp, \
             tc.tile_pool(name="sacc_h0", bufs=17) as sph0, \
             tc.tile_pool(name="sps", bufs=2, space="PSUM") as spsp:
            with tc.tile_pool(name="qkv_ps", bufs=2, space="PSUM") as qps:
                # m0 projections, then head 0 scores+exp (deferred AV)
                proj_qk(qps, wq_c, bq_sb, qT_sb, 0)
                h0_eh = []
                qTh0 = qT_sb[0][0:64, :]
                kTh0 = kT_sb[0][0:64, :]
                # interleave K-m0 chunk evacs with the kt quarters that
                # consume them: scores for kt 0-3 only need K chunk 0
                for ch4 in range(4):
                    proj_qk_chunk(qps, wk_c, bk_sb, kT_sb, 0, ch4)
                    for kt in range(ch4 * 4, ch4 * 4 + 4):
                        h0_eh.append(scores_exp(spsp, ep, sph0, kTh0, qTh0,
                                                kt, sacc_tag=f"_{kt}"))
                # m1 projections + V run on PE under head 0's exps
                proj_qk(qps, wq_c, bq_sb, qT_sb, 1)
                proj_qk(qps, wk_c, bk_sb, kT_sb, 1)
                for st in range(ST):
                    pv = qps.tile([128, DL], F32, tag="pv")
                    for c in range(CT):
                        nc.tensor.matmul(
                            pv[:],
                            xt_ch[c][:, st * 128:(st + 1) * 128],
                            wv_c[c][:],
                            start=(c == 0), stop=(c == CT - 1))
                    nc.vector.tensor_add(v_sb[:, st, :], pv[:], bvb[:])

            with tc.tile_pool(name="wps", bufs=1, space="PSUM") as wpsp:
                # pair 0: head 0's deferred AV interleaved with head 1
                wps = wpsp.tile([128, S], F32, tag="wps")
                qTh1 = qT_sb[0][64:128, :]
                kTh1 = kT_sb[0][64:128, :]
                for kt in range(ST):
                    vs0 = norm_vs(sp, h0_eh[kt], kt, 0 * HD, tag="0")
                    av(wps, vs0, h0_eh[kt], kt, 0, None)
                    eh1 = scores_exp(spsp, ep, sp, kTh1, qTh1, kt)
                    vs1 = norm_vs(sp, eh1, kt, 1 * HD, tag="1")
                    av(wps, vs1, eh1, kt, 64, (0, 64))
                for ch in range(4):
                    nc.vector.tensor_copy(
                        wt_sb[0][:, ch * 512:(ch + 1) * 512],
                        wps[:, ch * 512:(ch + 1) * 512])

                # pair 1: normal interleaved loop
                wps = wpsp.tile([128, S], F32, tag="wps")
                for sub in range(2):
                    h = 2 + sub
                    off = 64 * sub
                    qTh = qT_sb[1][off:off + 64, :]
                    kTh = kT_sb[1][off:off + 64, :]
                    tp = (0, 64 * sub) if sub else None
                    for kt in range(ST):
                        eh = scores_exp(spsp, ep, sp, kTh, qTh, kt)
                        vs = norm_vs(sp, eh, kt, h * HD)
                        av(wps, vs, eh, kt, off, tp)
                for ch in range(4):
                    nc.vector.tensor_copy(
                        wt_sb[1][:, ch * 512:(ch + 1) * 512],
                        wps[:, ch * 512:(ch + 1) * 512])

        # ---- output projection (partial over local heads) ----
        with tc.tile_pool(name="out_sb", bufs=6) as osb, \
             tc.tile_pool(name="ops", bufs=4, space="PSUM") as ops:
            for st in range(ST):
                po = ops.tile([128, D], F32, tag="po")
                for ch in range(2):
                    for c in range(2):
                        nc.tensor.matmul(
                            po[:, ch * 512:(ch + 1) * 512],
                            wt_sb[c][:, st * 128:(st + 1) * 128],
                            woT_sb[:, c, ch * 512:(ch + 1) * 512],
                            start=(c == 0), stop=(c == 1))
                ob = osb.tile([128, D], BF, tag="ob")
                if st % 2 == 0:
                    nc.vector.tensor_copy(ob[:], po[:])
                else:
                    nc.scalar.copy(ob[:], po[:])
                dq = nc.sync if st % 2 == 0 else nc.scalar
                dq.dma_start(out[st * 128:(st + 1) * 128, :], ob[:])


def _build(reps=None, marker=False, loop_n=None):
    """reps=None: single-shot kernel. reps=N: python-unrolled N repetitions
    of the whole body (benchmarking only). loop_n=N: hardware For_i loop
    around the body (benchmarking only). marker adds a dummy input named
    by reps so differently-unrolled builds can't alias in any compile cache."""
    nc = bacc.Bacc("TRN2", target_bir_lowering=False, debug=False,
                   num_devices=NCORES)
    if marker:
        nc.dram_tensor(f"repmark{loop_n or 0}_{reps or 1}", [1, 1], F32,
                       kind="ExternalInput")
    xT = nc.dram_tensor("xT", [D, S], BF, kind="ExternalInput").ap()
    wqT = nc.dram_tensor("wqT", [D, DL], BF, kind="ExternalInput").ap()
    wkT = nc.dram_tensor("wkT", [D, DL], BF, kind="ExternalInput").ap()
    wvT = nc.dram_tensor("wvT", [D, DL], BF, kind="ExternalInput").ap()
    woT = nc.dram_tensor("woT", [DL, D], BF, kind="ExternalInput").ap()
    bq = nc.dram_tensor("bq", [128, 2], F32, kind="ExternalInput").ap()
    bk = nc.dram_tensor("bk", [128, 2], F32, kind="ExternalInput").ap()
    bv = nc.dram_tensor("bv", [1, DL], F32, kind="ExternalInput").ap()
    out = nc.dram_tensor("out", [S, D], BF, kind="ExternalOutput").ap()
    aps = (xT, wqT, wkT, wvT, woT, bq, bk, bv, out)

    with tile.TileContext(nc) as tc:
        if loop_n is not None:
            hints = (mybir.EngineType.PE, mybir.EngineType.DVE,
                     mybir.EngineType.Activation, mybir.EngineType.SP,
                     mybir.EngineType.Pool)
            with tc.For_i(0, loop_n, 1, hint_engines=hints):
                _emit_body(nc, tc, aps)
        else:
            for _ in range(reps or 1):
                _emit_body(nc, tc, aps)

    nc.compile()
    return nc


def _get_nc():
    if "nc" not in _CACHE:
        _CACHE["nc"] = _build()
    return _CACHE["nc"]


def _make_in_maps(x, wq, bq, wk, bk, wv, bv, wo):
    xTs = [np.ascontiguousarray(x[b].T).astype(bf16) for b in range(B)]
    in_maps = []
    for core in range(NCORES):
        b, hg = core // (NCORES // B), core % (NCORES // B)
        rows = slice(hg * DL, (hg + 1) * DL)
        in_maps.append({
            "xT": xTs[b],
            "wqT": np.ascontiguousarray(wq[rows].T).astype(bf16),
            "wkT": np.ascontiguousarray(wk[rows].T).astype(bf16),
            "wvT": np.ascontiguousarray(wv[rows].T).astype(bf16),
            "woT": np.ascontiguousarray(wo[:, rows].T).astype(bf16),
            "bq": np.ascontiguousarray(bq[rows].reshape(2, 128).T),
            "bk": np.ascontiguousarray(bk[rows].reshape(2, 128).T),
            "bv": np.ascontiguousarray(bv[rows].reshape(1, DL)),
        })
    return in_maps


def kernel(x, wq, bq, wk, bk, wv, bv, wo, bo):
    global LAST_RESULT
    x = np.asarray(x, dtype=np.float32)
    wq, bq = np.asarray(wq, np.float32), np.asarray(bq, np.float32)
    wk, bk = np.asarray(wk, np.float32), np.asarray(bk, np.float32)
    wv, bv = np.asarray(wv, np.float32), np.asarray(bv, np.float32)
    wo, bo = np.asarray(wo, np.float32), np.asarray(bo, np.float32)

    nc = _get_nc()
    in_maps = _make_in_maps(x, wq, bq, wk, bk, wv, bv, wo)

    trace = os.environ.get("MHA_TRACE", "0") == "1"
    res = run_bass_kernel_spmd(nc, in_maps, core_ids=list(range(NCORES)),
                               trace=trace)
    LAST_RESULT = res

    out = np.zeros((B, S, D), np.float32)
    for core in range(NCORES):
        out[core // (NCORES // B)] += res.results[core]["out"].astype(np.float32)
    out += bo[None, None, :]
    return out



# revision 8
# speedup vs baseline: 1.0091x; 1.0091x over previous
"""MultiHeadAttention (B=2, S=2048, D=1024, H=16, softmax over query axis)
on 8 TRN2 NeuronCores.

Sharding: core c handles batch b = c//4 and head-group hg = c%4 (4 heads,
d_local = 256). QKV weights row-sharded by head group, Wo column-sharded;
each core produces a partial [S, D] output, host sums the 4 partials per
batch and adds the output bias.

Device kernel (per core), all matmuls bf16 with fp32 PSUM accumulation:
  xT [D, S] resident in SBUF (per-128-row tiles, DMA'd after the weights so
  projection matmuls pipeline with the input DMA).
  Q^T, K^T = Wl^T-stationary matmuls -> [d_local, S] (+bias, bf16)
  V = xT-stationary matmul -> [S, d_local] (+bias, bf16)
  per head: scores^T[k, q] = K^T-slice.T @ Q^T (contract d=64)
            softmax over q = free-axis Exp + accum_out row sums (fp32)
            normalizer folded into V rows: vs = V[k,:]*r[k]
            W^T[d, q] += vs.T @ exp(scores^T)   (contract k)
  out[s, j] = W^T-slices.T @ Wo^T  (contract d_local)
"""

import os

import numpy as np
import ml_dtypes

import concourse.tile as tile
from concourse import bacc, mybir
from concourse.bass_utils import run_bass_kernel_spmd

B, S, D, H = 2, 2048, 1024, 16
HD = D // H            # 64
NCORES = 8
HPC = H // (NCORES // B)   # heads per core = 4
DL = HPC * HD              # local head dims = 256
CT = D // 128              # 8 contraction tiles over D
ST = S // 128              # 16 seq tiles
BF = mybir.dt.bfloat16
F32 = mybir.dt.float32
bf16 = ml_dtypes.bfloat16

_CACHE = {}
LAST_RESULT = None


def _emit_body(nc, tc, aps):
    xT, wqT, wkT, wvT, woT, bq, bk, bv, out = aps
    with tc.tile_pool(name="const", bufs=1) as cp:
        # --- biases first (tiny), then weights+x interleaved per c-tile so
        # the c-chain of projection matmuls starts as soon as possible.
        bq_sb = cp.tile([128, 2], F32)
        bk_sb = cp.tile([128, 2], F32)
        nc.sync.dma_start(bq_sb[:], bq)
        nc.sync.dma_start(bk_sb[:], bk)
        bv_row = cp.tile([1, DL], F32)
        nc.sync.dma_start(bv_row[:], bv)
        bvb = cp.tile([128, DL], F32)
        nc.gpsimd.partition_broadcast(bvb[:], bv_row[:])

        wq_c, wk_c, wv_c, xt_ch = [], [], [], []
        for c in range(CT):
            cslice = slice(c * 128, (c + 1) * 128)
            tq = cp.tile([128, DL], BF, tag=f"wq{c}", name=f"wq{c}")
            tk = cp.tile([128, DL], BF, tag=f"wk{c}", name=f"wk{c}")
            tv = cp.tile([128, DL], BF, tag=f"wv{c}", name=f"wv{c}")
            nc.sync.dma_start(tq[:], wqT[cslice, :])
            nc.sync.dma_start(tk[:], wkT[cslice, :])
            nc.sync.dma_start(tv[:], wvT[cslice, :])
            tx = cp.tile([128, S], BF, tag=f"xt{c}", name=f"xt{c}")
            nc.scalar.dma_start(tx[:], xT[cslice, :])
            xt_ch.append(tx)
            wq_c.append(tq)
            wk_c.append(tk)
            wv_c.append(tv)

        woT_sb = cp.tile([128, DL // 128, D], BF)
        nc.scalar.dma_start(woT_sb[:], woT.rearrange("(c p) j -> p c j", p=128))

        qT_sb = [cp.tile([128, S], BF, tag=f"qT{m}", name=f"qT{m}")
                 for m in range(2)]
        kT_sb = [cp.tile([128, S], BF, tag=f"kT{m}", name=f"kT{m}")
                 for m in range(2)]
        v_sb = cp.tile([128, ST, DL], BF)
        wt_sb = [cp.tile([128, S], BF, tag=f"wt{m}", name=f"wt{m}")
                 for m in range(2)]

        # ---- QKV projections + attention, overlapped ----
        # The scores psum pool opens BEFORE the projection pool (LIFO stack)
        # so head 0's scores+exp chain starts right after the m0 projections
        # while the m1/V projections still run on PE; head 0's A*V matmuls
        # are deferred (E tiles retained) until V exists and the projection
        # pool's banks have been handed to the W accumulator pool.
        def scores_exp(spsp, ep, sp, kTh, qTh, kt, sacc_tag=""):
            e_half = []
            for qh in range(2):
                sps = spsp.tile([128, 1024], F32, tag="sps")
                for ch in range(2):
                    q0 = qh * 1024 + ch * 512
                    nc.tensor.matmul(
                        sps[:, ch * 512:(ch + 1) * 512],
                        kTh[:, kt * 128:(kt + 1) * 128],
                        qTh[:, q0:q0 + 512],
                        start=True, stop=True)
                e_t = ep.tile([128, 1024], BF, tag="E")
                sacc = sp.tile([128, 1], F32, tag=f"sacc{qh}{sacc_tag}")
                nc.scalar.activation(
                    e_t[:], sps[:], mybir.ActivationFunctionType.Exp,
                    scale=float(1.0 / np.sqrt(HD)), accum_out=sacc[:])
                e_half.append((e_t, sacc))
            return e_half

        def norm_vs(sp, e_half, kt, vcol, tag=""):
            stot = sp.tile([128, 1], F32, tag=f"stot{tag}")
            nc.vector.tensor_add(stot[:], e_half[0][1][:], e_half[1][1][:])
            r = sp.tile([128, 1], F32, tag=f"r{tag}")
            nc.vector.reciprocal(r[:], stot[:])
            vs = sp.tile([128, HD], BF, tag=f"vs{tag}")
            nc.vector.tensor_scalar_mul(
                vs[:], v_sb[:, kt, vcol:vcol + HD], r[:])
            return vs

        def av(wps, vs, e_half, kt, off, tp):
            for qh in range(2):
                for ch in range(2):
                    q0 = qh * 1024 + ch * 512
                    nc.tensor.matmul(
                        wps[off:off + 64, q0:q0 + 512],
                        vs[:], e_half[qh][0][:, ch * 512:(ch + 1) * 512],
                        start=(kt == 0), stop=(kt == ST - 1),
                        tile_position=tp)

        def proj_qk_chunk(qps, wc, bsb, dst, m, ch):
            pq = qps.tile([128, 512], F32, tag="pq")
            for c in range(CT):
                nc.tensor.matmul(
                    pq[:],
                    wc[c][:, m * 128:(m + 1) * 128],
                    xt_ch[c][:, ch * 512:(ch + 1) * 512],
                    start=(c == 0), stop=(c == CT - 1))
            nc.vector.tensor_scalar_add(
                dst[m][:, ch * 512:(ch + 1) * 512], pq[:], bsb[:, m:m + 1])

        def proj_qk(qps, wc, bsb, dst, m):
            for ch in range(4):
                pq = qps.tile([128, 512], F32, tag="pq")
                for c in range(CT):
                    nc.tensor.matmul(
                        pq[:],
                        wc[c][:, m * 128:(m + 1) * 128],
                        xt_ch[c][:, ch * 512:(ch + 1) * 512],
                        start=(c == 0), stop=(c == CT - 1))
                nc.vector.tensor_scalar_add(
                    dst[m][:, ch * 512:(ch + 1) * 512], pq[:],
                    bsb[:, m:m + 1])

        with tc.tile_pool(name="att_sb", bufs=36) as ep, \
             tc.tile_pool(name="small", bufs=4) as sp, \
             tc.tile_pool(name="sacc_h0", bufs=17) as sph0, \
             tc.tile_pool(name="sps", bufs=2, space="PSUM") as spsp:
            with tc.tile_pool(name="qkv_ps", bufs=2, space="PSUM") as qps:
                # m0 projections, then head 0 scores+exp (deferred AV)
                proj_qk(qps, wq_c, bq_sb, qT_sb, 0)
                h0_eh = []
                qTh0 = qT_sb[0][0:64, :]
                kTh0 = kT_sb[0][0:64, :]
                # interleave K-m0 chunk evacs with the kt quarters that
                # consume them: scores for kt 0-3 only need K chunk 0
                for ch4 in range(4):
                    proj_qk_chunk(qps, wk_c, bk_sb, kT_sb, 0, ch4)
                    for kt in range(ch4 * 4, ch4 * 4 + 4):
                        h0_eh.append(scores_exp(spsp, ep, sph0, kTh0, qTh0,
                                                kt, sacc_tag=f"_{kt}"))
                # m1 projections + V run on PE under head 0's exps
                proj_qk(qps, wq_c, bq_sb, qT_sb, 1)
                proj_qk(qps, wk_c, bk_sb, kT_sb, 1)
                for st in range(ST):
                    pv = qps.tile([128, DL], F32, tag="pv")
                    for c in range(CT):
                        nc.tensor.matmul(
                            pv[:],
                            xt_ch[c][:, st * 128:(st + 1) * 128],
                            wv_c[c][:],
                            start=(c == 0), stop=(c == CT - 1))
                    nc.vector.tensor_add(v_sb[:, st, :], pv[:], bvb[:])

            with tc.tile_pool(name="wps", bufs=1, space="PSUM") as wpsp:
                # pair 0: head 0's deferred AV interleaved with head 1
                wps = wpsp.tile([128, S], F32, tag="wps")
                qTh1 = qT_sb[0][64:128, :]
                kTh1 = kT_sb[0][64:128, :]
                for kt in range(ST):
                    vs0 = norm_vs(sp, h0_eh[kt], kt, 0 * HD, tag="0")
                    av(wps, vs0, h0_eh[kt], kt, 0, None)
                    eh1 = scores_exp(spsp, ep, sp, kTh1, qTh1, kt)
                    vs1 = norm_vs(sp, eh1, kt, 1 * HD, tag="1")
                    av(wps, vs1, eh1, kt, 64, (0, 64))
                for ch in range(4):
                    nc.vector.tensor_copy(
                        wt_sb[0][:, ch * 512:(ch + 1) * 512],
                        wps[:, ch * 512:(ch + 1) * 512])

                # pair 1: normal interleaved loop
                wps = wpsp.tile([128, S], F32, tag="wps")
                for sub in range(2):
                    h = 2 + sub
                    off = 64 * sub
                    qTh = qT_sb[1][off:off + 64, :]
                    kTh = kT_sb[1][off:off + 64, :]
                    tp = (0, 64 * sub) if sub else None
                    for kt in range(ST):
                        eh = scores_exp(spsp, ep, sp, kTh, qTh, kt)
                        vs = norm_vs(sp, eh, kt, h * HD)
                        av(wps, vs, eh, kt, off, tp)
                for ch in range(4):
                    nc.vector.tensor_copy(
                        wt_sb[1][:, ch * 512:(ch + 1) * 512],
                        wps[:, ch * 512:(ch + 1) * 512])

        # ---- output projection (partial over local heads) ----
        with tc.tile_pool(name="out_sb", bufs=6) as osb, \
             tc.tile_pool(name="ops", bufs=4, space="PSUM") as ops:
            for st in range(ST):
                po = ops.tile([128, D], F32, tag="po")
                for ch in range(2):
                    for c in range(2):
                        nc.tensor.matmul(
                            po[:, ch * 512:(ch + 1) * 512],
                            wt_sb[c][:, st * 128:(st + 1) * 128],
                            woT_sb[:, c, ch * 512:(ch + 1) * 512],
                            start=(c == 0), stop=(c == 1))
                ob = osb.tile([128, D], BF, tag="ob")
                if st % 2 == 0:
                    nc.vector.tensor_copy(ob[:], po[:])
                else:
                    nc.scalar.copy(ob[:], po[:])
                dq = nc.sync if st % 2 == 0 else nc.scalar
                dq.dma_start(out[st * 128:(st + 1) * 128, :], ob[:])


def _build(reps=None, marker=False, loop_n=None):
    """reps=None: single-shot kernel. reps=N: python-unrolled N repetitions
    of the whole body (benchmarking only). loop_n=N: hardware For_i loop
    around the body (benchmarking only). marker adds a dummy input named
    by reps so differently-unrolled builds can't alias in any compile cache."""
    nc = bacc.Bacc("TRN2", target_bir_lowering=False, debug=False,
                   num_devices=NCORES)
    if marker:
        nc.dram_tensor(f"repmark{loop_n or 0}_{reps or 1}", [1, 1], F32,
                       kind="ExternalInput")
    xT = nc.dram_tensor("xT", [D, S], BF, kind="ExternalInput").ap()
    wqT = nc.dram_tensor("wqT", [D, DL], BF, kind="ExternalInput").ap()
    wkT = nc.dram_tensor("wkT", [D, DL], BF, kind="ExternalInput").ap()
    wvT = nc.dram_tensor("wvT", [D, DL], BF, kind="ExternalInput").ap()
    woT = nc.dram_tensor("woT", [DL, D], BF, kind="ExternalInput").ap()
    bq = nc.dram_tensor("bq", [128, 2], F32, kind="ExternalInput").ap()
    bk = nc.dram_tensor("bk", [128, 2], F32, kind="ExternalInput").ap()
    bv = nc.dram_tensor("bv", [1, DL], F32, kind="ExternalInput").ap()
    out = nc.dram_tensor("out", [S, D], BF, kind="ExternalOutput").ap()
    aps = (xT, wqT, wkT, wvT, woT, bq, bk, bv, out)

    with tile.TileContext(nc) as tc:
        if loop_n is not None:
            hints = (mybir.EngineType.PE, mybir.EngineType.DVE,
                     mybir.EngineType.Activation, mybir.EngineType.SP,
                     mybir.EngineType.Pool)
            with tc.For_i(0, loop_n, 1, hint_engines=hints):
                _emit_body(nc, tc, aps)
        else:
            for _ in range(reps or 1):
                _emit_body(nc, tc, aps)

    nc.compile()
    return nc


def _get_nc():
    if "nc" not in _CACHE:
        _CACHE["nc"] = _build()
    return _CACHE["nc"]


def _make_in_maps(x, wq, bq, wk, bk, wv, bv, wo):
    xTs = [np.ascontiguousarray(x[b].T).astype(bf16) for b in range(B)]
    in_maps = []
    for core in range(NCORES):
        b, hg = core // (NCORES // B), core % (NCORES // B)
        rows = slice(hg * DL, (hg + 1) * DL)
        in_maps.append({
            "xT": xTs[b],
            "wqT": np.ascontiguousarray(wq[rows].T).astype(bf16),
            "wkT": np.ascontiguousarray(wk[rows].T).astype(bf16),
            "wvT": np.ascontiguousarray(wv[rows].T).astype(bf16),
            "woT": np.ascontiguousarray(wo[:, rows].T).astype(bf16),
            "bq": np.ascontiguousarray(bq[rows].reshape(2, 128).T),
            "bk": np.ascontiguousarray(bk[rows].reshape(2, 128).T),
            "bv": np.ascontiguousarray(bv[rows].reshape(1, DL)),
        })
    return in_maps


def kernel(x, wq, bq, wk, bk, wv, bv, wo, bo):
    global LAST_RESULT
    x = np.asarray(x, dtype=np.float32)
    wq, bq = np.asarray(wq, np.float32), np.asarray(bq, np.float32)
    wk, bk = np.asarray(wk, np.float32), np.asarray(bk, np.float32)
    wv, bv = np.asarray(wv, np.float32), np.asarray(bv, np.float32)
    wo, bo = np.asarray(wo, np.float32), np.asarray(bo, np.float32)

    nc = _get_nc()
    in_maps = _make_in_maps(x, wq, bq, wk, bk, wv, bv, wo)

    trace = os.environ.get("MHA_TRACE", "0") == "1"
    res = run_bass_kernel_spmd(nc, in_maps, core_ids=list(range(NCORES)),
                               trace=trace)
    LAST_RESULT = res

    out = np.zeros((B, S, D), np.float32)
    for core in range(NCORES):
        out[core // (NCORES // B)] += res.results[core]["out"].astype(np.float32)
    out += bo[None, None, :]
    return out

